# revision 8
# baseline (speedup 1.0000x reference)
"""Trainium2 Bass kernel for a single-head BERT attention (B=8, S=2048, E=1024, H=64).

Sharding: data-parallel over batch - one batch element per NeuronCore (8 cores).
Weights replicated. No collectives.

Per-core structure (all matmuls bf16 with fp32 PSUM accumulation):
  - x arrives as xT chunks laid out [128, chunk, s] and is streamed in 4
    s-quarters of 512 columns (1 MB DMAs), so score work starts ~4us in.
  - qk-proj per quarter: qkT[0:64]=q^T, qkT[64:128]=k^T accumulate over the 8
    E-chunks into one PSUM bank, then cast to bf16 SBUF.
  - v-proj in transposed form (stationary Wv chunks -> vT [64, s]), then
    per-tile DMA xbar transposes give natural v tiles [128, 64+1] with an
    appended ones column (so mm2 emits the softmax denominator for free).
  - mm1 (scores^T = k^T.T @ q^T, K=64) runs even/odd t-tiles concurrently on
    PE row-halves via tile_position; each pair's two PSUM banks are exp'd by
    ONE wide ScalarE activation into an interleaved pt_pair layout.
  - mm2 in transposed form: stationary v_j [128, 65], moving pt -> ctx^T[65, s]
    accumulated bank-by-bank (b-outer, j-inner); each bank is copied out and
    DMA'd as soon as its last contribution lands.
  - The kernel returns raw ctx^T [65, S] (row 64 = denominator); the host
    divides and transposes (cheap numpy) - identical math to on-device divide.

The program is specialized from the actual mask contents (block map reduced
over batch), so any mask is handled correctly; for the causal mask this halves
the score/exp/context work. A short burst of warmup matmuls on the weight
tiles runs at t~1us to lift the PE HAM clock-gate before real work arrives.
"""

import numpy as np
import ml_dtypes

import concourse.bass as bass  # noqa: F401  (import registers bass machinery)
import concourse.bacc as bacc
import concourse.mybir as mybir
import concourse.tile as tile
from concourse.bass_utils import run_bass_kernel_spmd

BF16 = ml_dtypes.bfloat16
B, S, E, H = 8, 2048, 1024, 64
P = 128          # partitions / tile edge
NS = S // P      # 16 seq tiles
NE = E // P      # 8 embed chunks
SB = 512         # one fp32 PSUM bank
NQ = S // SB     # 4 s-quarters

_cache: dict = {}
last_results = None  # BassKernelResults of the most recent run (for test harness)


def _plan_from_mask(mask: np.ndarray):
    """Derive the static block plan from the actual mask input.

    Returns (ranges, mask_items, full_cover, maskT):
      ranges[j]    = (lo, hi) element range of s that t-tile j must compute (or None)
      mask_items   = [(j, i)] 128x128 blocks inside range needing an elementwise
                     mask multiply (mixed or all-zero-in-range)
      full_cover[b]= True if ctx bank b's first contributor covers the full bank
    Valid for every batch element simultaneously (classifications reduced over batch).
    """
    m = np.asarray(mask, dtype=bool)
    mt = np.ascontiguousarray(m.transpose(0, 2, 1))  # [B, t, s]
    blocks = mt.reshape(B, NS, P, NS, P)
    any_ = blocks.any(axis=(2, 4))   # [B, tj, si]
    all_ = blocks.all(axis=(2, 4))
    nz = any_.any(axis=0)            # not all-zero in some batch -> must compute
    allone = all_.all(axis=0)        # all-ones in every batch -> no mask needed

    ranges = []
    for j in range(NS):
        cols = np.nonzero(nz[j])[0]
        if len(cols) == 0:
            ranges.append(None)
            continue
        ranges.append((int(cols.min()) * P, (int(cols.max()) + 1) * P))

    mask_items = []
    for j in range(NS):
        if ranges[j] is None:
            continue
        lo, hi = ranges[j]
        for i in range(lo // P, hi // P):
            if not allone[j, i]:
                mask_items.append((j, i))

    full_cover = []
    for b in range(NQ):
        cov = False
        for j in range(NS):
            if ranges[j] is None:
                continue
            lo, hi = ranges[j]
            if lo <= b * SB and hi >= (b + 1) * SB:
                cov = True
                break
        full_cover.append(cov)
    return ranges, mask_items, full_cover, mt


def _build_nc(ranges, mask_items, full_cover, has_bqk, has_bv):
    dt = mybir.dt
    n_mb = max(len(mask_items), 1)
    nc = bacc.Bacc("TRN2", target_bir_lowering=False, debug=False, num_devices=8)

    xr_d = nc.dram_tensor("xr", [P, NE * S], dt.bfloat16, kind="ExternalInput").ap()
    wqk_d = nc.dram_tensor("wqk", [P, NE * 2 * H], dt.bfloat16, kind="ExternalInput").ap()
    wv_d = nc.dram_tensor("wv", [P, NE * H], dt.bfloat16, kind="ExternalInput").ap()
    bqk_d = nc.dram_tensor("bqk", [1, 2 * H], dt.bfloat16, kind="ExternalInput").ap()
    bv_d = nc.dram_tensor("bv", [1, H], dt.bfloat16, kind="ExternalInput").ap()
    mb_d = nc.dram_tensor("maskb", [P, n_mb * P], dt.bfloat16, kind="ExternalInput").ap()
    id_d = nc.dram_tensor("ident", [64, 64], dt.bfloat16, kind="ExternalInput").ap()
    y_d = nc.dram_tensor("yT", [H + 1, S], dt.float32, kind="ExternalOutput").ap()

    xr = xr_d.rearrange("p (q c s) -> p q c s", q=NQ, c=NE)

    # per-(bank, j) mm2 stream range
    def mm2_range(b, j):
        if ranges[j] is None:
            return None
        lo, hi = ranges[j]
        lo2, hi2 = max(lo, b * SB), min(hi, (b + 1) * SB)
        if lo2 >= hi2:
            return None
        return lo2, hi2

    mm2_js = [[j for j in range(NS) if mm2_range(b, j) is not None] for b in range(NQ)]

    # mm1 pair col-ranges per quarter: for pair jp, quarter q -> (lo, hi) union
    def mm1_pair_range(q, jp):
        los, his = [], []
        for j in (2 * jp, 2 * jp + 1):
            if ranges[j] is None:
                continue
            lo, hi = ranges[j]
            lo2, hi2 = max(lo, q * SB), min(hi, (q + 1) * SB)
            if lo2 < hi2:
                los.append(lo2)
                his.append(hi2)
        if not los:
            return None
        return min(los), max(his)

    EXP = mybir.ActivationFunctionType.Exp
    with tile.TileContext(nc) as tc:
        with (
            tc.tile_pool(name="consts", bufs=1) as cpool,
            tc.tile_pool(name="xt", bufs=1) as xpool,
            tc.tile_pool(name="qk", bufs=1) as qkpool,
            tc.tile_pool(name="vex", bufs=1) as vpool,
            tc.tile_pool(name="pt", bufs=1) as ppool,
            tc.tile_pool(name="maskp", bufs=1) as mpool,
            tc.tile_pool(name="outs", bufs=2) as opool,
            tc.tile_pool(name="projps", bufs=1, space="PSUM") as projpsum,
            tc.tile_pool(name="wps", bufs=1, space="PSUM") as wpsum,
            tc.tile_pool(name="cps", bufs=1, space="PSUM") as cpsum,
        ):
            # ---- constants ----
            wqk_sb = cpool.tile([P, NE, 2 * H], dt.bfloat16)
            nc.gpsimd.dma_start(wqk_sb[:], wqk_d.rearrange("p (c h) -> p c h", c=NE))
            wv_sb = cpool.tile([P, NE, H], dt.bfloat16)
            nc.gpsimd.dma_start(wv_sb[:], wv_d.rearrange("p (c h) -> p c h", c=NE))
            bqk_sb = cpool.tile([1, 2 * H], dt.bfloat16)
            bv_sb = cpool.tile([1, H], dt.bfloat16)
            if has_bqk:
                nc.gpsimd.dma_start(bqk_sb[:], bqk_d[:])
            if has_bv:
                nc.gpsimd.dma_start(bv_sb[:], bv_d[:])
            id_sb = cpool.tile([64, 64], dt.bfloat16)
            nc.scalar.dma_start(id_sb[:], id_d[:])
            ones_sb = cpool.tile([1, SB], dt.bfloat16)
            nc.vector.memset(ones_sb[:], 1.0)
            warm_sb = cpool.tile([1, 2], dt.float32)
            nc.scalar.activation(warm_sb[:], ones_sb[0:1, 0:2], EXP, scale=0.125)

            # ---- mask blocks: one packed DMA, sliced per block ----
            mask_all = mpool.tile([P, n_mb * P], dt.bfloat16, name="mask_all")
            nc.scalar.dma_start(mask_all[:], mb_d[:])
            mask_tiles = {}
            for idx, (j, i) in enumerate(mask_items):
                mask_tiles[(j, i)] = mask_all[:, idx * P:(idx + 1) * P]

            # ---- HAM warmup: rank-1 junk matmuls with no input DMA deps,
            # into a psum bank that mm2 reuses much later (WAR-safe).
            warm_ps = cpsum.tile([P, 2, SB], dt.float32, tag="cps", name="warm_ps")
            for w in range(6):
                nc.tensor.matmul(warm_ps[:, 0, 0:448], ones_sb[:, 0:P],
                                 ones_sb[:, 0:448], start=True, stop=True)

            # ---- persistent SBUF tensors ----
            xq = [xpool.tile([P, NE, SB], dt.bfloat16, tag=f"xq{q}", name=f"xq{q}")
                  for q in range(NQ)]
            qkT_sb = qkpool.tile([P, S], dt.bfloat16)
            kT_sb = qkpool.tile([64, S], dt.bfloat16)
            qhi_sb = qkpool.tile([P, S], dt.bfloat16)
            vt_sb = qkpool.tile([64, S], dt.bfloat16)
            vext = [vpool.tile([P, 4, H + 1], dt.bfloat16, tag=f"vx{q}", name=f"vx{q}")
                    for q in range(NQ)]
            for q in range(NQ):
                nc.vector.memset(vext[q][:, :, H:H + 1], 1.0)
            # pt_pair[jp]: [P, q, parity, SB] interleaved exp(scores^T) storage
            ptp = [ppool.tile([P, NQ, 2, SB], dt.bfloat16, tag=f"pt{jp}",
                              name=f"pt{jp}") for jp in range(NS // 2)]

            def emit_quarter_proj(qs):
                for q in qs:
                    if q == 0:
                        nc.sync.dma_start(xq[0][:, 0:4, :], xr[:, 0, 0:4, :])
                        nc.sync.dma_start(xq[0][:, 4:8, :], xr[:, 0, 4:8, :])
                    else:
                        nc.sync.dma_start(xq[q][:], xr[:, q, :, :])
                qk_ps = {q: projpsum.tile([P, SB], dt.float32, tag=f"qv{q % 2}",
                                          name="qk_ps") for q in qs}
                for c in range(NE):
                    for q in qs:
                        nc.tensor.matmul(qk_ps[q][:], wqk_sb[:, c, :], xq[q][:, c, :],
                                         start=(c == 0),
                                         stop=(not has_bqk and c == NE - 1))
                for q in qs:
                    if has_bqk:
                        nc.tensor.matmul(qk_ps[q][:], bqk_sb[:], ones_sb[:],
                                         start=False, stop=True)
                    blk = slice(q * SB, (q + 1) * SB)
                    nc.vector.tensor_copy(qkT_sb[:, blk], qk_ps[q][:])
                    # partition fixups: k^T down to 0-63, q^T up to 64-127
                    fix = nc.scalar if q < 2 else nc.gpsimd
                    fix.dma_start(kT_sb[:, blk], qkT_sb[64:128, blk])
                    fix.dma_start(qhi_sb[64:128, blk], qkT_sb[0:64, blk])
                vt_ps = {q: projpsum.tile([64, SB], dt.float32, tag=f"qv{q % 2}",
                                          name="vt_ps") for q in qs}
                for c in range(NE):
                    for q in qs:
                        nc.tensor.matmul(vt_ps[q][:], wv_sb[:, c, :], xq[q][:, c, :],
                                         start=(c == 0),
                                         stop=(not has_bv and c == NE - 1))
                for q in qs:
                    if has_bv:
                        nc.tensor.matmul(vt_ps[q][:], bv_sb[:], ones_sb[:],
                                         start=False, stop=True)
                    nc.vector.tensor_copy(vt_sb[:, q * SB:(q + 1) * SB], vt_ps[q][:])

            def emit_vtrans(q):
                tp = projpsum.tile([P, 4, H], dt.bfloat16, tag=f"qv{q % 2}", name="tp")
                for jj in range(4):
                    j = 4 * q + jj
                    nc.tensor.matmul(
                        tp[:, jj, :], vt_sb[:, j * P:(j + 1) * P], id_sb[:],
                        is_transpose=True, start=(jj == 0), stop=(jj == 3))
                nc.vector.tensor_copy(vext[q][:, :, 0:H], tp[:])

            def emit_mm1_quarter(qs):
                for jp in range(2 * max(qs) + 2):
                    prs = {q: mm1_pair_range(q, jp) for q in qs}
                    if all(pr is None for pr in prs.values()):
                        continue
                    je, jo = 2 * jp, 2 * jp + 1
                    ps = {q: wpsum.tile([P, 2, SB], dt.float32,
                                        tag=f"wps{(q + jp) % 2}", name="wps")
                          for q in qs if prs[q] is not None}
                    # even-j matmuls for all quarters (one stationary), then odd-j
                    if ranges[je] is not None:
                        for q in qs:
                            if prs[q] is None:
                                continue
                            lo, hi = prs[q]
                            nc.tensor.matmul(
                                ps[q][:, 0, 0:hi - lo],
                                kT_sb[:, je * P:(je + 1) * P], qkT_sb[0:64, lo:hi],
                                start=True, stop=True, tile_position=(0, 0))
                    if ranges[jo] is not None:
                        for q in qs:
                            if prs[q] is None:
                                continue
                            lo, hi = prs[q]
                            nc.tensor.matmul(
                                ps[q][:, 1, 0:hi - lo],
                                qkT_sb[64:128, jo * P:(jo + 1) * P],
                                qhi_sb[64:128, lo:hi],
                                start=True, stop=True, tile_position=(64, 0))
                    for q in qs:
                        if prs[q] is None:
                            continue
                        lo, hi = prs[q]
                        nc.scalar.activation(
                            ptp[jp][:, q, :, lo - q * SB:hi - q * SB],
                            ps[q][:, :, 0:hi - lo], EXP, scale=0.125)
                        # elementwise mask on not-all-ones blocks of this quarter
                        for j, par in ((je, 0), (jo, 1)):
                            for i in range(q * 4, q * 4 + 4):
                                if (j, i) in mask_tiles:
                                    off = (i % 4) * P
                                    sl = ptp[jp][:, q, par, off:off + P]
                                    nc.vector.tensor_mul(sl, sl, mask_tiles[(j, i)])

            def emit_mm2_pair(pb):
                b0, b1 = 2 * pb, 2 * pb + 1
                js = sorted(set(mm2_js[b0]) | set(mm2_js[b1]))
                ob = opool.tile([H + 1, 2, SB], dt.float32, tag="out", name=f"ob{pb}")
                if not js:
                    nc.vector.memset(ob[:], 0.0)
                    nc.gpsimd.dma_start(y_d[:, b0 * SB:(b1 + 1) * SB],
                                        ob.rearrange("p a s -> p (a s)"))
                    return
                pc = cpsum.tile([P, 2, SB], dt.float32, tag="cps", name="pc")
                for b in (b0, b1):
                    if not full_cover[b] and mm2_js[b]:
                        nc.vector.memset(pc[0:H + 1, b - b0, :], 0.0)
                started = {b0: full_cover[b0], b1: full_cover[b1]}
                last = {}
                for j in js:
                    for b in (b0, b1):
                        if mm2_range(b, j) is not None:
                            last[b] = j
                for j in js:
                    for b in (b0, b1):
                        r = mm2_range(b, j)
                        if r is None:
                            continue
                        lo, hi = r
                        first = started[b] and j == mm2_js[b][0]
                        nc.tensor.matmul(
                            pc[0:H + 1, b - b0, lo - b * SB:hi - b * SB],
                            vext[j // 4][:, j % 4, :],
                            ptp[j // 2][:, b, j % 2, lo - b * SB:hi - b * SB],
                            start=first, stop=(j == last[b]))
                for b in (b0, b1):
                    if not mm2_js[b]:
                        nc.vector.memset(pc[0:H + 1, b - b0, :], 0.0)
                nc.vector.tensor_copy(ob[:], pc[0:H + 1, :, :])
                nc.gpsimd.dma_start(y_d[:, b0 * SB:(b1 + 1) * SB],
                                    ob.rearrange("p a s -> p (a s)"))

            # ---- software pipeline over quarters ----
            emit_quarter_proj([0])
            emit_quarter_proj([1])
            emit_vtrans(0)
            emit_mm1_quarter([0])
            emit_quarter_proj([2, 3])
            emit_vtrans(1)
            emit_mm1_quarter([1])
            emit_vtrans(2)
            emit_vtrans(3)
            emit_mm2_pair(0)
            emit_mm1_quarter([2, 3])
            emit_mm2_pair(1)

    nc.compile()
    return nc


def kernel(x, mask, Wq, bq, Wk, bk, Wv, bv, _trace=False, _trace_kwargs=None):
    global last_results
    x = np.asarray(x, dtype=np.float32)
    ranges, mask_items, full_cover, maskT = _plan_from_mask(mask)

    has_bqk = bool(np.any(bq)) or bool(np.any(bk))
    has_bv = bool(np.any(bv))
    key = (tuple(ranges), tuple(mask_items), tuple(full_cover), has_bqk, has_bv)
    nc = _cache.get(key)
    if nc is None:
        nc = _build_nc(ranges, mask_items, full_cover, has_bqk, has_bv)
        _cache[key] = nc

    wqk = np.concatenate([np.asarray(Wq), np.asarray(Wk)], axis=1)
    wqk = np.ascontiguousarray(
        wqk.reshape(NE, P, 2 * H).transpose(1, 0, 2)).reshape(P, NE * 2 * H).astype(BF16)
    wv = np.ascontiguousarray(
        np.asarray(Wv).reshape(NE, P, H).transpose(1, 0, 2)).reshape(P, NE * H).astype(BF16)
    bqk = np.concatenate([np.asarray(bq), np.asarray(bk)])[None, :].astype(BF16)
    bvv = np.asarray(bv)[None, :].astype(BF16)

    in_maps = []
    for b in range(B):
        # xr[p, q, c, s] = x[b][512q+s, 128c+p]
        xr_b = np.ascontiguousarray(
            x[b].T.reshape(NE, P, NQ, SB).transpose(1, 2, 0, 3)
        ).reshape(P, NQ * NE * SB).astype(BF16)
        if mask_items:
            mb = np.concatenate([
                maskT[b, j * P:(j + 1) * P, i * P:(i + 1) * P]
                for (j, i) in mask_items], axis=1).astype(BF16)
        else:
            mb = np.zeros((P, P), dtype=BF16)
        in_maps.append({
            "xr": xr_b, "wqk": wqk, "wv": wv, "bqk": bqk, "bv": bvv, "maskb": mb,
            "ident": np.eye(64, dtype=BF16),
        })

    res = run_bass_kernel_spmd(
        nc, in_maps, core_ids=list(range(B)),
        trace=_trace, **(_trace_kwargs or {}))
    last_results = res
    out = np.empty((B, S, H), dtype=np.float32)
    for b in range(B):
        yT = res.results[b]["yT"]
        denom = yT[H]
        safe = np.where(denom == 0.0, 1.0, denom)
        out[b] = (yT[0:H] / safe).T
    return out
